# revision 19
# baseline (speedup 1.0000x reference)
"""CAM (channel attention) module kernel for Trainium2 (Bass/Tile).

Reference computation (per batch b):
    energy  = x_b @ x_b.T                      # [C, C], contraction over N
    att     = softmax(rowmax(energy) - energy) # row-wise over last axis
    out     = att @ x_b                        # [C, N]
    y_b     = gamma * out + x_b

Sharding: data-parallel over B across 8 NeuronCores (B=32 -> 4 per core),
gamma replicated, full CxC attention per core.

Identity used: softmax(rowmax(E) - E)[i,j] = exp(mn[i] - E[i,j]) / Z[i]
with mn[i] = min_j E[i,j], Z[i] = sum_j exp(mn[i] - E[i,j])  (shift
invariance of softmax; exact).

Layouts per batch (P=128 partitions):
    X   [P, CO, N]  c-natural  (c = co*P + p)           -- DMA from DRAM
    X16 [P, CO, N]  bf16 copy (matmul-2 moving operand)  -- GpSimd cast
    xT  [P, C]      per k-chunk, n on partitions         -- PE transpose, f32r
    E   [P, CO, C]  PSUM, i on partitions, j on free     -- matmul 1 (f32r)
    t   [P, CO, C]  SBUF f32, exp(mn - E), Z fused       -- ScalarE activation
    tT  [P, CO, C]  SBUF bf16, j on partitions           -- PE transpose
    out chunk [P, 512] = (tT.T @ X16) * (gamma/Z[i]) + X -- matmul 2 + DVE

`reps` wraps the whole body in a hardware loop (identical work each
iteration, static addressing) -- used only for timing runs.
"""

import contextlib

import numpy as np

P = 128

_CACHE = {}


def _build(Bs, C, N, use_f32r=True, reps=1):
    import concourse.bass as bass  # noqa: F401
    import concourse.tile as tile
    import concourse.mybir as mybir
    from concourse import bacc
    from concourse.masks import make_identity

    F32 = mybir.dt.float32
    BF16 = mybir.dt.bfloat16
    MMDT = mybir.dt.float32r if use_f32r else mybir.dt.bfloat16
    AF = mybir.ActivationFunctionType
    ALU = mybir.AluOpType
    AX = mybir.AxisListType

    assert C == 4 * P and N % 512 == 0
    CO = C // P          # i/j chunks of 128
    KC = N // P          # n chunks of 128 (contraction for energy)
    NF = N // 512        # n chunks of 512 (matmul-2 free dim)

    nc = bacc.Bacc(None, target_bir_lowering=False, debug=False)
    x_in = nc.dram_tensor("x", [Bs, C, N], F32, kind="ExternalInput")
    g_in = nc.dram_tensor("gamma", [1], F32, kind="ExternalInput")
    y_out = nc.dram_tensor("y", [Bs, C, N], F32, kind="ExternalOutput")

    with tile.TileContext(nc) as tc:
        with (
            tc.tile_pool(name="consts", bufs=1) as consts,
            tc.tile_pool(name="xpool", bufs=2) as xpool,
            tc.tile_pool(name="x16pool", bufs=1) as x16pool,
            tc.tile_pool(name="xtp", bufs=3) as xtp,
            tc.tile_pool(name="tpool", bufs=1) as tpool,
            tc.tile_pool(name="ttpool", bufs=2) as ttpool,
            tc.tile_pool(name="opool", bufs=4) as opool,
            tc.tile_pool(name="stats", bufs=2) as stats,
            tc.tile_pool(name="pe", bufs=1, space="PSUM") as psum_e,
            tc.tile_pool(name="pxt", bufs=2, space="PSUM") as psum_xt,
            tc.tile_pool(name="pacc", bufs=2, space="PSUM") as psum_acc,
        ):
            ident = consts.tile([P, P], F32)
            make_identity(nc, ident)
            g_sb = consts.tile([1, 1], F32)
            nc.sync.dma_start(g_sb[:, :], g_in[:].rearrange("(a b) -> a b", a=1))
            g_col = consts.tile([P, 1], F32)
            nc.gpsimd.partition_broadcast(g_col[:, :], g_sb[:1, :1])

            loop_ctx = (
                tc.For_i(0, reps, 1) if reps > 1 else contextlib.nullcontext()
            )
            with loop_ctx:
                for b in range(Bs):
                    x_b = x_in[b].rearrange("(co p) n -> p co n", p=P)
                    y_b = y_out[b].rearrange("(co p) n -> p co n", p=P)

                    X = xpool.tile([P, CO, N], F32, tag="X")
                    for nf in range(NF):
                        s = slice(nf * 512, (nf + 1) * 512)
                        nc.sync.dma_start(X[:, :, s], x_b[:, :, s])

                    # bf16 copy of x for matmul-2's moving operand
                    X16 = x16pool.tile([P, CO, N], BF16, tag="X16")
                    for co in range(CO):
                        nc.gpsimd.tensor_copy(X16[:, co, :], X[:, co, :])

                    # ---- energy = x @ x.T (contraction over n on partitions)
                    E = psum_e.tile([P, CO, C], F32, tag="E")
                    for kc in range(KC):
                        ks = slice(kc * P, (kc + 1) * P)
                        ps_x = psum_xt.tile([P, C], F32, tag="psx")
                        for co in range(CO):
                            nc.tensor.transpose(
                                ps_x[:, co * P:(co + 1) * P], X[:, co, ks], ident
                            )
                        xt_k = xtp.tile([P, C], MMDT, tag="xt")
                        nc.scalar.copy(xt_k[:, :], ps_x[:, :])
                        for ic in range(CO):
                            nc.tensor.matmul(
                                E[:, ic, :],
                                xt_k[:, ic * P:(ic + 1) * P],
                                xt_k[:, :],
                                start=(kc == 0),
                                stop=(kc == KC - 1),
                            )

                    # ---- softmax: t = exp(mn - E), Z row-sum fused ----
                    mn = stats.tile([P, CO], F32, tag="mn")
                    zs = stats.tile([P, CO], F32, tag="zs")
                    rg = stats.tile([P, CO], F32, tag="rg")
                    tS = tpool.tile([P, CO, C], F32, tag="t")
                    for ic in range(CO):
                        nc.vector.tensor_reduce(
                            mn[:, ic:ic + 1], E[:, ic, :], AX.X, ALU.min
                        )
                    for ic in range(CO):
                        nc.scalar.activation(
                            tS[:, ic, :], E[:, ic, :], AF.Exp,
                            bias=mn[:, ic:ic + 1], scale=-1.0,
                            accum_out=zs[:, ic:ic + 1],
                        )
                    nc.vector.reciprocal(rg[:, :], zs[:, :])
                    nc.vector.tensor_scalar_mul(rg[:, :], rg[:, :], g_col[:, :1])

                    # ---- tT[j, i] = t[i, j] via PE transpose ----
                    tT = ttpool.tile([P, CO, C], BF16, tag="tT")
                    for jc in range(CO):
                        ps_t = psum_acc.tile([P, C], F32, tag="acc")
                        for ic in range(CO):
                            nc.tensor.transpose(
                                ps_t[:, ic * P:(ic + 1) * P],
                                tS[:, ic, jc * P:(jc + 1) * P],
                                ident,
                            )
                        nc.scalar.copy(tT[:, jc, :], ps_t[:, :])

                    # ---- out = att @ x, scaled by gamma/Z + residual ----
                    for ic in range(CO):
                        for nf in range(NF):
                            ns = slice(nf * 512, (nf + 1) * 512)
                            ps2 = psum_acc.tile([P, C], F32, tag="acc")
                            for jc in range(CO):
                                nc.tensor.matmul(
                                    ps2[:, :512],
                                    tT[:, jc, ic * P:(ic + 1) * P],
                                    X16[:, jc, ns],
                                    start=(jc == 0),
                                    stop=(jc == CO - 1),
                                )
                            o = opool.tile([P, 512], F32, tag="o")
                            nc.vector.scalar_tensor_tensor(
                                o[:, :], ps2[:, :512], rg[:, ic:ic + 1],
                                X[:, ic, ns],
                                op0=ALU.mult, op1=ALU.add,
                            )
                            nc.sync.dma_start(y_b[:, ic, ns], o[:, :])

    nc.compile()
    return nc


def get_nc(Bs=4, C=512, N=4096, use_f32r=True, reps=1):
    key = (Bs, C, N, use_f32r, reps)
    if key not in _CACHE:
        _CACHE[key] = _build(*key)
    return _CACHE[key]


def kernel(x, gamma):
    """Full inputs in, full output out. x [32, 512, 4096] f32, gamma [1] f32."""
    from concourse.bass_utils import run_bass_kernel_spmd

    x = np.ascontiguousarray(np.asarray(x, dtype=np.float32))
    gamma = np.ascontiguousarray(np.asarray(gamma, dtype=np.float32))
    B, C, N = x.shape
    n_cores = 8
    assert B % n_cores == 0
    Bs = B // n_cores

    nc = get_nc(Bs, C, N)
    in_maps = [
        {"x": x[i * Bs:(i + 1) * Bs], "gamma": gamma} for i in range(n_cores)
    ]
    res = run_bass_kernel_spmd(nc, in_maps, core_ids=list(range(n_cores)))
    return np.concatenate([r["y"] for r in res.results], axis=0)
